# revision 40
# baseline (speedup 1.0000x reference)
"""Multi-head self-attention (B=4, S=1024, D=1024, H=16, RoPE, causal) on 8
Trainium2 NeuronCores.

Sharding: 8 cores = 4 batches x 2 head-groups (8 heads each). Each core
computes QKV projections for its batch/head-group, RoPE, causal attention,
and a partial output projection (contraction over its 512 attention dims).
The host sums the two partial outputs per batch (the "all-reduce") and
concatenates batches.

Device layout notes:
- Weights are passed transposed and merged ([wq|wk|wv] along the output
  dim); Q/K projection output dims are permuted to rotate-half order
  (evens then odds within each head) so RoPE works on contiguous
  32-column halves. Permuting Q and K identically leaves Q.K^T unchanged.
- Logits are computed transposed (L^T[k, q]); both 512-col chunks of a
  k-tile land in one 2-bank PSUM tile so a single ACT exp covers them.
  128-wide diagonal chunks are widened to 256 (f32r matmuls need a
  >=256 moving dim for 1 cyc/row); the widened below-diagonal prefix is
  zeroed by the same gpsimd mask multiply that applies the causal
  triangle.
- A ones-column in V makes the last EV output row the softmax sums. For
  odd heads the ones-column comes FIRST and the EV output is written at
  partition offset 63, so attn rows land at partitions 64..127 directly.
- Softmax normalization is all on-chip: DVE reciprocal of the sums row,
  PE rank-1 ones-matmul broadcasts it across partitions, DVE copies it
  to SBUF, and gpsimd multiplies the (already SBUF-copied) raw attn
  in place. Copying raw attn out of PSUM right after EV releases the
  PSUM slot early so heads pipeline without stalls.
- Matmul operands are float32r (1 cyc/row at >=256 moving dim).
- The final output is produced transposed (y^T[o, q]); the host transposes.
"""

import numpy as np

import concourse.bass as bass
import concourse.mybir as mybir
import concourse.tile as tile
from concourse.bass import ts
from concourse.bass_utils import run_bass_kernel_spmd
from concourse.masks import make_identity, make_upper_triangular

B, S, D = 4, 1024, 1024
H = 16  # total heads
HG = 8  # heads per core (head-group)
DK = 64  # head dim
DG = HG * DK  # 512, per-core projection width
ROPE_THETA = 10000.0
P = 128  # partitions
NS = S // P  # 8 s-tiles
ND = D // P  # 8 d-chunks
F32 = mybir.dt.float32
F32R = mybir.dt.float32r
MMD = F32R  # matmul operand dtype

_uid = [0]


def _split_excess_waits(nc, limit=1):
    """This container's walrus rejects >1 sync waits on the kernel-tail
    Drain; move excess waits onto same-engine NoOps inserted before it."""
    for f in nc.m.functions:
        for blk in f.blocks:
            insts = list(blk.instructions)
            out = []
            changed = False
            for inst in insts:
                si = inst.sync_info
                if si is not None and si.on_wait and len(si.on_wait) > limit:
                    waits = list(si.on_wait)
                    head, tail = waits[:-limit], waits[-limit:]
                    for i in range(0, len(head), limit):
                        _uid[0] += 1
                        nop = mybir.InstNoOp(
                            name=f"waitsplit-{_uid[0]}", ins=[], outs=[]
                        )
                        nop.engine = inst.engine
                        nop.sync_info = mybir.SyncInfo(
                            on_wait=head[i : i + limit], on_update=[]
                        )
                        out.append(nop)
                    si.on_wait = tail
                    changed = True
                out.append(inst)
            if changed:
                blk.instructions = out
    return nc


def build_nc():
    nc = bass.Bass("TRN2")
    MMD = F32R  # matmul operand dtype
    xT = nc.dram_tensor("xT", [D, S], MMD, kind="ExternalInput")
    wT = nc.dram_tensor("wT", [D, 3 * DG], MMD, kind="ExternalInput")
    woT = nc.dram_tensor("woT", [DG, D], MMD, kind="ExternalInput")
    cossin = nc.dram_tensor("cossin", [S, 2 * HG * 32], F32, kind="ExternalInput")
    yT = nc.dram_tensor("yT", [D, S], F32, kind="ExternalOutput")
    # DRAM scratch for the softmax 1/sum reshape + partition-broadcast bounce
    rsum = nc.dram_tensor("rsum", [HG, S], F32)
    rrec = nc.dram_tensor("rrec", [HG, S], F32)

    with tile.TileContext(nc) as tc:
        with (
            tc.tile_pool(name="const", bufs=1) as constp,
            tc.tile_pool(name="big", bufs=1) as bigp,
        ):
            # constants (f32r tiles can't be memset directly; build in f32
            # and convert via ACT copy, which rounds)
            identf = constp.tile([P, P], F32, tag="identf")
            make_identity(nc, identf[:, :])
            ident = constp.tile([P, P], MMD, tag="ident")
            nc.scalar.copy(out=ident[:, :], in_=identf[:, :])
            # ztril: [zeros(384) | lower-keep causal mask(128)] — sliced per
            # k-tile to zero both the widened below-diagonal prefix and the
            # above-diagonal part of the diagonal block
            ztril = constp.tile([P, 512], F32, tag="ztril")
            nc.vector.memset(ztril[:, :], 0.0)
            make_upper_triangular(nc, ztril[:, 384:512], val=1.0, diag=True)
            onesf = constp.tile([P, DK], F32, tag="onesf")
            nc.vector.memset(onesf[:, :], 1.0)
            zerof = constp.tile([P, S], F32, tag="zerof")
            nc.vector.memset(zerof[:, :], 0.0)
            assert HG <= DK

            # persistent activations. kt is per-head, zero-padded to K=128
            # (the other head's d-rows are zero) and v is padded to M=128
            # so every attention matmul is a uniform K=128/M=128 FWL-
            # eligible shape — mixed shapes break the PE's LDWEIGHTS
            # pull-ahead and cost ~300ns per matmul.
            qt_sb = [bigp.tile([P, S], MMD, tag=f"qt{p}", name=f"qt{p}") for p in range(4)]
            kt_z = [bigp.tile([P, S], MMD, tag=f"kz{h}", name=f"kz{h}") for h in range(HG)]
            v_sb = [bigp.tile([P, HG, P], MMD, tag=f"v{j}", name=f"v{j}") for j in range(NS)]

            # ---------------- Phase A: projections + RoPE + transposes ----
            with (
                tc.tile_pool(name="pa_psum", bufs=2, space="PSUM") as pap,
                tc.tile_pool(name="tp_psum", bufs=2, space="PSUM") as tpp,
                tc.tile_pool(name="wq", bufs=1) as wqp,
                tc.tile_pool(name="pa_sbuf", bufs=4) as pas,
                tc.tile_pool(name="rope", bufs=3) as ropep,
            ):
                w_all = wqp.tile([P, ND, 3 * DG], MMD, tag="w", name="w_all")

                def load_weights():
                    # on the scalar HWDGE queue so x/cos loads (sync queue)
                    # aren't stuck behind 6 MB of weights
                    for c in range(ND):
                        nc.scalar.dma_start(
                            out=w_all[:, c, :], in_=wT[ts(c, P), :]
                        )

                load_weights()
                # zero the padding halves of the per-head kt tiles
                for h in range(HG):
                    rows = slice(DK, P) if h % 2 == 0 else slice(0, DK)
                    eng = nc.scalar if h % 2 == 0 else nc.vector
                    (eng.copy if h % 2 == 0 else eng.tensor_copy)(
                        out=kt_z[h][rows, :], in_=zerof[rows, :]
                    )
                # (v_sb's M-padding columns stay uninitialized: the
                # garbage only reaches EV output rows 65..127, never read)
                def emit_transposes(i, rmap):
                    # transpose head-pairs into [d, s] tiles; copies split
                    # across ACT and DVE. k goes to per-head zero-padded
                    # tiles (head 2p rows 0:64, head 2p+1 rows 64:128).
                    # Emitted one s-tile late so the PE doesn't idle
                    # waiting for this tile's RoPE chain.
                    for dst_tag, r in rmap.items():
                        for p in range(4):
                            tp = tpp.tile([P, P], MMD, tag="tp")
                            nc.tensor.transpose(
                                tp[:, :], r[:, ts(p, P)], ident[:, :]
                            )
                            eng_copy = (
                                nc.scalar.copy
                                if p % 2 == 0
                                else nc.vector.tensor_copy
                            )
                            if dst_tag == "qr":
                                eng_copy(
                                    out=qt_sb[p][:, ts(i, P)], in_=tp[:, :]
                                )
                            else:
                                eng_copy(
                                    out=kt_z[2 * p][0:DK, ts(i, P)],
                                    in_=tp[0:DK, :],
                                )
                                eng_copy(
                                    out=kt_z[2 * p + 1][DK:P, ts(i, P)],
                                    in_=tp[DK:P, :],
                                )

                pending_tp = None
                for i in range(NS):
                    xt_all = pas.tile([P, ND, P], MMD, tag="xt", name="xt")
                    nc.sync.dma_start(
                        out=xt_all[:, :, :],
                        in_=xT[:, :].rearrange("(c p) s -> p c s", p=P)[
                            :, :, ts(i, P)
                        ],
                    )
                    xt = [xt_all[:, c, :] for c in range(ND)]
                    cs = pas.tile([P, 512], F32, tag="cos")
                    nc.sync.dma_start(out=cs[:, :], in_=cossin[ts(i, P), :])

                    # one accumulation chain per output (constant PSUM
                    # bank per chain — interleaving q/k/v cycles PSUM
                    # banks every matmul and throttles the PE), and q/k
                    # finish earlier so RoPE starts sooner
                    qp = pap.tile([P, DG], F32, tag="q")
                    kp = pap.tile([P, DG], F32, tag="k")
                    vp = pap.tile([P, DG], F32, tag="v")
                    for g, dst in enumerate((qp, kp, vp)):
                        for c in range(ND):
                            nc.tensor.matmul(
                                dst[:, :], lhsT=xt[c],
                                rhs=w_all[:, c, g * DG : (g + 1) * DG],
                                start=(c == 0), stop=(c == ND - 1),
                            )

                    if pending_tp is not None:
                        emit_transposes(*pending_tp)
                        pending_tp = None

                    # V -> SBUF with a ones column per head (softmax sums)
                    nc.scalar.copy(
                        out=v_sb[i][:, :, 0:DK],
                        in_=vp[:, :].rearrange("p (h c) -> p h c", h=HG),
                    )
                    nc.scalar.copy(
                        out=v_sb[i][:, :, DK : DK + 1],
                        in_=onesf[:, 0:HG].rearrange("p (h c) -> p h c", c=1),
                    )

                    # RoPE on q/k (rotate-half layout: per head [32 even|32 odd])
                    cs3 = cs[:, 0:256].rearrange("p (h c) -> p h c", h=HG)
                    sn3 = cs[:, 256:512].rearrange("p (h c) -> p h c", h=HG)
                    rmap = {}
                    for src, dst_tag in ((qp, "qr"), (kp, "kr")):
                        sv = src[:, :].rearrange(
                            "p (h t c) -> p h t c", h=HG, t=2
                        )
                        ev, od = sv[:, :, 0, :], sv[:, :, 1, :]
                        r = ropep.tile([P, DG], MMD, tag=dst_tag, name=dst_tag)
                        rv = r[:, :].rearrange("p (h t c) -> p h t c", h=HG, t=2)
                        t1 = ropep.tile([P, HG * 32], F32, tag=dst_tag + "t1")
                        t2 = ropep.tile([P, HG * 32], F32, tag=dst_tag + "t2")
                        t13 = t1[:, :].rearrange("p (h c) -> p h c", h=HG)
                        t23 = t2[:, :].rearrange("p (h c) -> p h c", h=HG)
                        nc.vector.tensor_mul(t13, ev, cs3)
                        nc.vector.tensor_mul(t23, od, sn3)
                        nc.gpsimd.tensor_sub(rv[:, :, 0, :], t13, t23)
                        t3 = ropep.tile([P, HG * 32], F32, tag=dst_tag + "t3")
                        t4 = ropep.tile([P, HG * 32], F32, tag=dst_tag + "t4")
                        t33 = t3[:, :].rearrange("p (h c) -> p h c", h=HG)
                        t43 = t4[:, :].rearrange("p (h c) -> p h c", h=HG)
                        nc.vector.tensor_mul(t33, ev, sn3)
                        nc.vector.tensor_mul(t43, od, cs3)
                        nc.gpsimd.tensor_add(rv[:, :, 1, :], t33, t43)
                        rmap[dst_tag] = r
                    pending_tp = (i, rmap)
                emit_transposes(*pending_tp)

            # ---------------- Phase B: attention per head ------------------
            # Per k-tile j the two 512-col logits chunks share one 2-bank
            # PSUM tile: one fused exp per j. EV is emitted ~2 k-tiles
            # behind QK. Raw attn is copied out of PSUM right after the
            # EVs (releasing the ap slot); the softmax 1/sum broadcast
            # (DVE recip -> PE rank-1 -> DVE copy) and the gpsimd
            # normalize multiply run one head later, off the critical path.
            with (
                tc.tile_pool(name="wo", bufs=1) as wop,
                tc.tile_pool(name="at_pool", bufs=1) as atsp,
            ):
                wo_all = wop.tile([P, DG // P, D], MMD, tag="wo", name="wo_all")
                nc.scalar.dma_start(
                    out=wo_all[:, :, :],
                    in_=woT[:, :].rearrange("(c p) o -> p c o", p=P),
                )
                wo_sb = [wo_all[:, c, :] for c in range(DG // P)]
                at_sb = [
                    atsp.tile([P, S], MMD, tag=f"at{p}", name=f"at{p}")
                    for p in range(4)
                ]
                _phase_b(nc, tc, kt_z, qt_sb, v_sb, at_sb, ztril, rsum, rrec)
                _phase_d(nc, tc, wo_sb, at_sb, yT)

    _split_excess_waits(nc)
    return nc


def _phase_b(nc, tc, kt_z, qt_sb, v_sb, at_sb, ztril, rsum, rrec):
    with (
        tc.tile_pool(name="attn_psum", bufs=2, space="PSUM") as atp,
        tc.tile_pool(name="lg_psum", bufs=2, space="PSUM") as lgp,
        tc.tile_pool(name="pt_pool", bufs=4) as ptp,
        tc.tile_pool(name="rc_pool", bufs=2) as rcp,
        tc.tile_pool(name="bcs_pool", bufs=2) as bcsp,
    ):
                def emit_ev(h, ap, j, span_lo, pt):
                    st, sp = (j == 0), (j == NS - 1)
                    for ap_lo, ap_hi, pt_lo in EVS[j]:
                        nc.tensor.matmul(
                            ap[0:P, ap_lo:ap_hi],
                            lhsT=v_sb[j][:, h, :],
                            rhs=pt[:, pt_lo : pt_lo + ap_hi - ap_lo],
                            start=st, stop=sp, skip_group_check=True,
                        )

                def emit_tail(h, pair, odd, rc, dmaq=None):
                    # softmax 1/sum: bounce the sums row through DRAM to a
                    # [128, 8] layout (so the DVE reciprocal runs on 128
                    # lanes, not 1), bounce back, then broadcast across
                    # partitions with a stride-0 DRAM read and normalize
                    # the SBUF attn in place on DVE. All DMAs ride the
                    # otherwise-idle sync queue, one head behind the
                    # critical path.
                    dstart = DK if odd else 0
                    nc.sync.dma_start(
                        out=rsum[h, :].rearrange("(o c) -> o c", o=1),
                        in_=rc[DK : DK + 1, :],
                    )
                    rs8 = rcp.tile([P, NS], F32, tag="rs8", name=f"rs8{h}")
                    nc.sync.dma_start(
                        out=rs8[:, :],
                        in_=rsum[h, :].rearrange("(p c) -> p c", p=P),
                    )
                    rc8 = rcp.tile([P, NS], F32, tag="rc8", name=f"rc8{h}")
                    nc.vector.reciprocal(out=rc8[:, :], in_=rs8[:, :])
                    nc.sync.dma_start(
                        out=rrec[h, :].rearrange("(p c) -> p c", p=P),
                        in_=rc8[:, :],
                    )
                    row = rrec[h, :]
                    bc_src = bass.AP(
                        tensor=row.tensor,
                        offset=row.offset,
                        ap=[[0, DK], [1, S]],
                    )
                    bcs = bcsp.tile([P, S], F32, tag="bcs", name=f"bcs{h}")
                    nc.sync.dma_start(
                        out=bcs[dstart : dstart + DK, :], in_=bc_src
                    )
                    nc.vector.tensor_mul(
                        at_sb[pair][dstart : dstart + DK, :],
                        at_sb[pair][dstart : dstart + DK, :],
                        bcs[dstart : dstart + DK, :],
                    )

                # per k-tile j: QK chunk placement in the lg tile
                # (lg_lo, lg_hi, q_lo), causal-mask window in pt-col space
                # (mask_lo, mask_hi), and EV slices (ap_lo, ap_hi, pt_lo).
                # j4+j5 and j6+j7 share one lg tile / one exp.
                QKS = {
                    0: [(0, 512, 0), (512, 1024, 512)],
                    1: [(128, 512, 128), (512, 1024, 512)],
                    2: [(256, 512, 256), (512, 1024, 512)],
                    3: [(256, 512, 256), (512, 1024, 512)],
                    4: [(512, 1024, 512)],
                    5: [(0, 384, 640)],
                    6: [(0, 256, 768)],
                    7: [(256, 512, 768)],
                }
                MASKS = {
                    0: (0, 128), 1: (128, 256), 2: (256, 384), 3: (256, 512),
                    4: (512, 640), 5: (0, 128), 6: (0, 128), 7: (256, 512),
                }
                EVS = {
                    0: [(0, 512, 0), (512, 1024, 512)],
                    1: [(128, 512, 0), (512, 1024, 384)],
                    2: [(256, 512, 0), (512, 1024, 256)],
                    3: [(256, 512, 0), (512, 1024, 256)],
                    4: [(512, 1024, 512)],
                    5: [(640, 1024, 0)],
                    6: [(768, 1024, 0)],
                    7: [(768, 1024, 256)],
                }
                GROUPS = [((0,), 0, 1024), ((1,), 128, 1024),
                          ((2,), 256, 1024), ((3,), 256, 1024),
                          ((4, 5), 0, 1024), ((6, 7), 0, 512)]

                state = {"tail": None, "epi": None}

                def flush_epilogue(last=False):
                    # final EVs for the previous head, then copy its sums
                    # row and raw attn to SBUF (freeing the ap slot);
                    # normalization happens in the tail, later. Odd heads
                    # land at partitions 64..127 of at_sb via an
                    # SBUF->SBUF DMA.
                    h, pair, odd, ap, pending = state["epi"]
                    state["epi"] = None
                    for args in pending:
                        emit_ev(h, ap, *args)
                    rc = rcp.tile([P, S], F32, tag="rc", name=f"rc{h}")
                    rc_copy = (
                        nc.scalar.copy if last else nc.vector.tensor_copy
                    )
                    rc_copy(out=rc[DK : DK + 1, :], in_=ap[DK : DK + 1, :])
                    if not odd:
                        nc.vector.tensor_copy(
                            out=at_sb[pair][0:DK, :], in_=ap[0:DK, :]
                        )
                    else:
                        tmp = rcp.tile([P, S], MMD, tag="odd", name=f"odd{h}")
                        nc.vector.tensor_copy(
                            out=tmp[0:DK, :], in_=ap[0:DK, :]
                        )
                        nc.sync.dma_start(
                            out=at_sb[pair][DK:P, :], in_=tmp[0:DK, :]
                        )
                    state["tail"] = (h, pair, odd, rc)

                for h in (6, 7, 1, 0, 2, 3, 5, 4):
                    pair, odd = divmod(h, 2)
                    ap = atp.tile([P, S], F32, tag="attn", name=f"ap{h}")
                    pending = []
                    for gi, (grp, span_lo, span_hi) in enumerate(GROUPS):
                        lg = lgp.tile([P, S], F32, tag="lg", name="lg")
                        for j in grp:
                            for lg_lo, lg_hi, q_lo in QKS[j]:
                                nc.tensor.matmul(
                                    lg[:, lg_lo:lg_hi],
                                    lhsT=kt_z[h][:, ts(j, P)],
                                    rhs=qt_sb[pair][
                                        :, q_lo : q_lo + lg_hi - lg_lo
                                    ],
                                    start=True, stop=True,
                                )
                        if gi == 0 and state["epi"] is not None:
                            flush_epilogue()
                        if gi == 3 and state["tail"] is not None:
                            emit_tail(*state["tail"])
                            state["tail"] = None
                        pt = ptp.tile([P, S], MMD, tag="pt", name="pt")
                        nc.scalar.activation(
                            out=pt[:, 0 : span_hi - span_lo],
                            in_=lg[:, span_lo:span_hi],
                            func=mybir.ActivationFunctionType.Exp,
                            scale=0.125,
                        )
                        for j in grp:
                            m_lo, m_hi = MASKS[j]
                            m_lo, m_hi = m_lo - span_lo, m_hi - span_lo
                            mw = m_hi - m_lo
                            eng = nc.gpsimd if j % 4 == 0 else nc.vector
                            eng.tensor_mul(
                                pt[:, m_lo:m_hi],
                                pt[:, m_lo:m_hi],
                                ztril[:, 512 - mw : 512],
                            )
                            pending.append((j, span_lo, pt))
                            if len(pending) > 2:
                                emit_ev(h, ap, *pending.pop(0))
                    state["epi"] = (h, pair, odd, ap, pending)
                flush_epilogue(last=True)
                emit_tail(*state["tail"], dmaq=nc.scalar)


def _phase_d(nc, tc, wo_sb, at_sb, yT):
    # Output projection. The first four o-tiles accumulate c=0..2 before
    # any c=3 matmul is emitted, so the PE has ~6us of work to chew on
    # while the final head's softmax tail (which produces at_sb[3] rows
    # 64..127) drains.
    with (
        tc.tile_pool(name="d_psum", bufs=4, space="PSUM") as dp,
        tc.tile_pool(name="y_sbuf", bufs=3) as ys,
    ):
        # c accumulation order ends with pair 2 — the last head processed
        # in phase B — so its softmax tail has the whole prelude to drain
        CORDER = (3, 0, 1, 2)

        def emit_mm(yp, o, ci, qcs=(0, 512)):
            c = CORDER[ci]
            for qc in qcs:
                nc.tensor.matmul(
                    yp[:, qc : qc + 512],
                    lhsT=wo_sb[c][:, ts(o, P)],
                    rhs=at_sb[c][:, qc : qc + 512],
                    start=(ci == 0), stop=(ci == DG // P - 1),
                    skip_group_check=True,
                )

        def emit_out(yp, o):
            ysb = ys.tile([P, S], F32, tag="ysb", name="ysb")
            if o % 2 == 0:
                nc.scalar.copy(out=ysb[:, :], in_=yp[:, :])
            else:
                nc.vector.tensor_copy(out=ysb[:, :], in_=yp[:, :])
            nc.sync.dma_start(out=yT[ts(o, P), :], in_=ysb[:, :])

        yps = {}
        for o in range(4):
            yps[o] = dp.tile([P, S], F32, tag="yp", name=f"y{o}")
        for o in range(4):
            for ci in range(3):
                emit_mm(yps[o], o, ci, qcs=(0,))
            for ci in range(3):
                emit_mm(yps[o], o, ci, qcs=(512,))
        for o in range(4):
            emit_mm(yps[o], o, 3, qcs=(0,))
            emit_mm(yps[o], o, 3, qcs=(512,))
            emit_out(yps[o], o)
        for o in range(4, ND):
            yp = dp.tile([P, S], F32, tag="yp", name=f"y{o}")
            for ci in range(DG // P):
                emit_mm(yp, o, ci, qcs=(0,))
            for ci in range(DG // P):
                emit_mm(yp, o, ci, qcs=(512,))
            emit_out(yp, o)

    _split_excess_waits(nc)
    return nc


_NC_CACHE = {}


def _get_nc():
    if "nc" not in _NC_CACHE:
        _NC_CACHE["nc"] = build_nc()
    return _NC_CACHE["nc"]


# rotate-half permutation within each head: evens then odds
_PERM = np.concatenate([np.arange(0, DK, 2), np.arange(1, DK, 2)])


def _host_prep(x, Wq, Wk, Wv, Wo, token_positions):
    """Build the 8 per-core input dicts."""
    inv_freq = 1.0 / (ROPE_THETA ** (np.arange(0, DK, 2, dtype=np.float32) / DK))
    in_maps = []
    for core in range(8):
        b, g = core // 2, core % 2
        heads = np.arange(HG * g, HG * (g + 1))
        rows_qk = (heads[:, None] * DK + _PERM[None, :]).reshape(-1)
        rows_v = (heads[:, None] * DK + np.arange(DK)[None, :]).reshape(-1)
        pos = token_positions[b].astype(np.float32)  # [S]
        ang = pos[:, None] * inv_freq[None, :]  # [S, 32]
        cos8 = np.tile(np.cos(ang), (1, HG)).astype(np.float32)
        sin8 = np.tile(np.sin(ang), (1, HG)).astype(np.float32)
        wT = np.concatenate(
            [Wq[rows_qk, :].T, Wk[rows_qk, :].T, Wv[rows_v, :].T], axis=1
        )
        in_maps.append(
            {
                "xT": np.ascontiguousarray(x[b].T),
                "wT": np.ascontiguousarray(wT),
                "woT": np.ascontiguousarray(Wo[:, rows_v].T),
                "cossin": np.concatenate([cos8, sin8], axis=1),
            }
        )
    return in_maps


def kernel(x, Wq, Wk, Wv, Wo, token_positions, _trace=False):
    x = np.asarray(x, dtype=np.float32)
    Wq = np.asarray(Wq, dtype=np.float32)
    Wk = np.asarray(Wk, dtype=np.float32)
    Wv = np.asarray(Wv, dtype=np.float32)
    Wo = np.asarray(Wo, dtype=np.float32)
    token_positions = np.asarray(token_positions)

    nc = _get_nc()
    in_maps = _host_prep(x, Wq, Wk, Wv, Wo, token_positions)
    res = run_bass_kernel_spmd(nc, in_maps, core_ids=list(range(8)), trace=_trace)
    if _trace:
        kernel.last_exec_time_ns = res.exec_time_ns
        kernel.last_results = res

    y = np.empty((B, S, D), dtype=np.float32)
    for b in range(B):
        yT0 = res.results[2 * b]["yT"]
        yT1 = res.results[2 * b + 1]["yT"]
        y[b] = (yT0 + yT1).T
    return y
